# revision 1
# baseline (speedup 1.0000x reference)
# Trainium2 Bass kernel for nn_MixedFFN (B=8, T=2048, D=1024, F=4096, LNS=32).
#
# Sharding across 8 NeuronCores (no collectives needed):
#   - Shared-FFN branch (tokens 0..2015): core c handles batch row c.
#   - Per-position branch (last 32 token positions, distinct weights per
#     position): core c handles positions 4c..4c+3 for all 8 batch rows.
#
# Per-core math:
#   shared: ysT = (gelu(x_c @ W1S) @ W2S).T computed as
#     L1: hT[f, t] = sum_k W1S[k, f] * xT[k, t]      (lhsT = W1S tile, rhs = xT)
#         a = gelu(h) stored transposed [F, tokens] in fp16
#     L2: ysT[d, t] = sum_f W2S[f, d] * aT[f, t]     (lhsT = W2S tile, rhs = aT)
#   per-position q: h = x_pos @ W1NS[q] (M=8 batch rows), gelu, transpose on PE,
#     y = a @ W2NS[q] via lhsT = aT[f, b] and rhs = W2NS[q] tiles.
#
# All matmuls in fp16 (PE streams 1 col/cycle), PSUM accumulates fp32.
# Shared tokens processed exactly (2016 = 2 blocks x 2 chunks x 504), no pad.
# yst output is fp16 (host upcasts); ytp stays fp32.

import numpy as np

B, T, D, F, LNS = 8, 2048, 1024, 4096, 32
S = T - LNS            # 2016 shared tokens per batch row
NCORES = 8
QPC = LNS // NCORES    # 4 positions per core
NBLK = 2               # token blocks for the shared branch
BLK = S // NBLK        # 1008 tokens per block
CH = BLK // 2          # 504-token chunks (PE moving dim <= 512)
KD = D // 128          # 8  k-tiles over D
MF = F // 128          # 32 m-tiles over F
W1G = 8                # W1S column groups (4 m-tiles = 512 cols each, 1MB)
W1GW = F // W1G        # 512
NQ = 8                 # D-column tiles for L2 output (dq)
PPN1 = 8               # per-position L1 chunks over F (512 wide)
PPW1 = F // PPN1       # 512
PPN2 = 2               # per-position L2 chunks over D (512 wide)
PPW2 = D // PPN2       # 512

_CACHE = {}


def _build_nc(loop_n=0, parts="all"):
    """Build + bacc-compile the single-core SPMD program. Cached per process.

    loop_n > 0 wraps the whole body in a hardware For_i loop that repeats the
    kernel loop_n times inside one NEFF execution — a timing instrument only.
    parts: "all" | "shared" | "pp" — emit only a subset (timing experiments).
    """
    key = ("nc", loop_n, parts)
    if key in _CACHE:
        return _CACHE[key]

    import concourse.mybir as mybir
    import concourse.tile as tile
    from concourse import bacc
    from concourse.masks import make_identity

    f32 = mybir.dt.float32
    f16 = mybir.dt.float16
    GELU = mybir.ActivationFunctionType.Gelu

    nc = bacc.Bacc(None, target_bir_lowering=False)

    # ---- kernel I/O (per-core shapes; host packs these layouts) ----
    xt_d = nc.dram_tensor("xt", [128, KD, S], f16, kind="ExternalInput")
    w1_d = nc.dram_tensor("w1", [W1G, 128, KD, W1GW], f16, kind="ExternalInput")
    w2_d = nc.dram_tensor("w2", [NQ, 128, MF, 128], f16, kind="ExternalInput")
    w1n_d = nc.dram_tensor("w1n", [QPC, 128, PPN1, KD, PPW1], f16, kind="ExternalInput")
    w2n_d = nc.dram_tensor("w2n", [QPC, 128, PPN2, MF, PPW2], f16, kind="ExternalInput")
    xp_d = nc.dram_tensor("xp", [128, KD, B * QPC], f16, kind="ExternalInput")
    yst_d = nc.dram_tensor("yst", [D, S], f16, kind="ExternalOutput")
    ytp_d = nc.dram_tensor("ytp", [B * QPC, D], f32, kind="ExternalOutput")

    with tile.TileContext(nc) as tc:
        with (
            tc.tile_pool(name="constp", bufs=1) as constp,
            tc.tile_pool(name="xqp", bufs=2) as xqp,        # [128, 8, 1008] f16, 2MB
            tc.tile_pool(name="w1p", bufs=2) as w1p,        # [128, 8, 512] f16, 1MB
            tc.tile_pool(name="atp", bufs=1) as atp,        # [128, 32, 1008] f16, 8.3MB
            tc.tile_pool(name="w2p", bufs=2) as w2p,        # [128, 32, 128] f16, 1MB
            tc.tile_pool(name="w1np", bufs=3) as w1np,      # [128, 8, 512] f16, 1MB
            tc.tile_pool(name="w2np", bufs=3) as w2np,      # [128, 8, 512] f16, 1MB
            tc.tile_pool(name="xpp", bufs=1) as xpp,
            tc.tile_pool(name="appp", bufs=2) as appp,      # [8, 4096] f16
            tc.tile_pool(name="atppp", bufs=2) as atppp,    # [128, 32, 8] f16
            tc.tile_pool(name="outp", bufs=3) as outp,      # [128, 504] f16
            tc.tile_pool(name="ypp", bufs=2) as ypp,        # [8, 512] f32
            tc.tile_pool(name="psum", bufs=2, space="PSUM") as psum,
        ):
            import contextlib

            identity = constp.tile([128, 128], f16)
            make_identity(nc, identity)

            loop_cm = (
                tc.For_i(0, loop_n, 1, hint_engines=(mybir.EngineType.PE,))
                if loop_n
                else contextlib.nullcontext()
            )
            with loop_cm:
              xp_holder = []

              def emit_xp():
                  xp_sb = xpp.tile([128, KD, B * QPC], f16, name="xp_sb")
                  xp_holder.append(xp_sb)
                  nc.sync.dma_start(out=xp_sb[:], in_=xp_d[:])

              # ---------- emission units ----------
              # Shared L1 for one block: produce aT fp16 [128, MF, BLK]
              # First w1 chunk's DMA is issued before the (bigger) xq loads so
              # the PE can start as early as possible.
              def shared_l1_block(blk):
                  at_b = atp.tile([128, MF, BLK], f16, name=f"at_{blk}", tag="at")
                  w1g0 = w1p.tile([128, KD, W1GW], f16, name=f"w1g_{blk}_0", tag="w1g")
                  nc.sync.dma_start(out=w1g0[:], in_=w1_d[0])
                  xq = xqp.tile([128, KD, BLK], f16, name=f"xq_{blk}", tag="xq")
                  for tch in range(2):
                      nc.sync.dma_start(
                          out=xq[:, :, tch * CH : (tch + 1) * CH],
                          in_=xt_d[:, :, blk * BLK + tch * CH : blk * BLK + (tch + 1) * CH],
                      )
                  if blk == 0 and parts != "shared":
                      emit_xp()

                  def unit(g, hook):
                      if g == 0:
                          w1g = w1g0
                      else:
                          w1g = w1p.tile([128, KD, W1GW], f16, name=f"w1g_{blk}_{g}", tag="w1g")
                          nc.sync.dma_start(out=w1g[:], in_=w1_d[g])
                      for ml in range(4):
                          m = 4 * g + ml
                          for tch in range(2):
                              ps = psum.tile([128, CH], f32, tag="l1", bufs=2, name=f"psl1_{blk}_{m}_{tch}")
                              for k in range(KD):
                                  nc.tensor.matmul(
                                      ps[:],
                                      w1g[:, k, ml * 128 : (ml + 1) * 128],
                                      xq[:, k, tch * CH : (tch + 1) * CH],
                                      start=(k == 0),
                                      stop=(k == KD - 1),
                                  )
                              nc.scalar.activation(
                                  at_b[:, m, tch * CH : (tch + 1) * CH], ps[:], GELU
                              )
                              hook()

                  return at_b, [lambda hook, g=g: unit(g, hook) for g in range(W1G)]

              # Shared L2 for one block, one dq column: ysT tile [128, 504] x2
              def shared_l2_unit(blk, at_b, dq, hook):
                  w2g = w2p.tile([128, MF, 128], f16, name=f"w2g_{blk}_{dq}", tag="w2g")
                  nc.sync.dma_start(out=w2g[:], in_=w2_d[dq])
                  for tch in range(2):
                      ps = psum.tile([128, CH], f32, tag="l2", bufs=2, name=f"psl2_{blk}_{dq}_{tch}")
                      for k in range(MF):
                          nc.tensor.matmul(
                              ps[:],
                              w2g[:, k, :],
                              at_b[:, k, tch * CH : (tch + 1) * CH],
                              start=(k == 0),
                              stop=(k == MF - 1),
                          )
                      ot = outp.tile([128, CH], f16, name=f"ot_{blk}_{dq}_{tch}", tag="ot")
                      nc.vector.tensor_copy(ot[:], ps[:])
                      nc.sync.dma_start(
                          out=yst_d[
                              dq * 128 : (dq + 1) * 128,
                              blk * BLK + tch * CH : blk * BLK + (tch + 1) * CH,
                          ],
                          in_=ot[:],
                      )
                      hook()

              # Per-position L1 chunk: h [8, 512] -> gelu into app_q
              def pp_l1_unit(q, app_q, nch):
                  xp_sb = xp_holder[0]
                  w1t = w1np.tile([128, KD, PPW1], f16, name=f"w1n_{q}_{nch}", tag="w1n")
                  nc.sync.dma_start(out=w1t[:], in_=w1n_d[q, :, nch])
                  ps = psum.tile([8, 512], f32, tag="pp", bufs=2, name=f"pspp1_{q}_{nch}")
                  for k in range(KD):
                      nc.tensor.matmul(
                          ps[:],
                          xp_sb[:, k, q * B : (q + 1) * B],
                          w1t[:, k, :],
                          start=(k == 0),
                          stop=(k == KD - 1),
                      )
                  nc.scalar.activation(
                      app_q[:, nch * PPW1 : (nch + 1) * PPW1], ps[:], GELU
                  )

              # Per-position transposes: app_q [8, 4096] -> atpp_q [128, 32, 8]
              def pp_transpose_unit(q, app_q, atpp_q, f):
                  tp = psum.tile([128, 8], f16, tag="tp", bufs=2, name=f"pst_{q}_{f}")
                  nc.tensor.transpose(
                      tp[:], app_q[:, f * 128 : (f + 1) * 128], identity[:8, :8]
                  )
                  nc.vector.tensor_copy(atpp_q[:, f, :], tp[:])

              # Per-position L2 for (q, nch): y chunk [8, 512]
              def pp_l2_unit(q, atpp_q, nch):
                  ps = psum.tile([8, 512], f32, tag="pp", bufs=2, name=f"pspp2_{q}_{nch}")
                  for kg in range(4):
                      w2t = w2np.tile([128, 8, PPW2], f16, name=f"w2n_{q}_{nch}_{kg}", tag="w2n")
                      nc.sync.dma_start(
                          out=w2t[:], in_=w2n_d[q, :, nch, kg * 8 : (kg + 1) * 8, :]
                      )
                      for kk in range(8):
                          k = kg * 8 + kk
                          nc.tensor.matmul(
                              ps[:],
                              atpp_q[:, k, :],
                              w2t[:, kk, :],
                              start=(k == 0),
                              stop=(k == MF - 1),
                          )
                  yt = ypp.tile([8, 512], f32, name=f"yt_{q}_{nch}", tag="yt")
                  nc.vector.tensor_copy(yt[:], ps[:])
                  nc.sync.dma_start(
                      out=ytp_d[q * B : (q + 1) * B, nch * PPW2 : (nch + 1) * PPW2],
                      in_=yt[:],
                  )

              # ---------- emission schedule ----------
              # Side work (per-position branch) is interleaved between shared
              # units so its big weight DMAs overlap the shared-branch compute.
              # Front-loaded so the tail doesn't run DMA-bound on its own.
              side = []
              app_tiles = {}
              atpp_tiles = {}
              for q in range(QPC):
                  def mk_app(q=q):
                      app_tiles[q] = appp.tile([8, F], f16, name=f"app_{q}", tag="app")
                  side.append(mk_app)
                  for nch in range(PPN1):
                      side.append(lambda q=q, nch=nch: pp_l1_unit(q, app_tiles[q], nch))
                  def mk_atpp(q=q):
                      atpp_tiles[q] = atppp.tile([128, MF, 8], f16, name=f"atpp_{q}", tag="atpp")
                  side.append(mk_atpp)
                  for f in range(MF):
                      side.append(lambda q=q, f=f: pp_transpose_unit(q, app_tiles[q], atpp_tiles[q], f))
                  for nch in range(PPN2):
                      side.append(lambda q=q, nch=nch: pp_l2_unit(q, atpp_tiles[q], nch))

              side_i = 0

              def emit_side(n):
                  nonlocal side_i
                  if parts == "shared":
                      return
                  budget = n
                  while side_i < len(side) and budget > 0:
                      side[side_i]()
                      side_i += 1
                      budget -= 1

              # Side list length: 4 * (1 + 8 + 1 + 32 + 2) = 176 units.
              # Interleaved at psum-group granularity: a hook after each of
              # the 160 shared psum groups emits 1-2 side units (front-loaded
              # x2 early on) so per-position DMAs/activations spread out and
              # never gate the PE for long.
              if parts in ("all", "shared"):
                  group_i = 0

                  def hook():
                      nonlocal group_i
                      group_i += 1
                      emit_side(2 if group_i <= 48 else 1)

                  for blk in range(NBLK):
                      at_b, l1_units = shared_l1_block(blk)
                      for g, u in enumerate(l1_units):
                          u(hook)
                      for dq in range(NQ):
                          shared_l2_unit(blk, at_b, dq, hook)
              else:
                  emit_xp()
              emit_side(len(side))

    nc.compile()
    _CACHE[key] = nc
    return nc


def pack_inputs(x, W1S, W2S, W1NS, W2NS):
    """Build the 8 per-core input maps (numpy, host-side layout packing)."""
    x = np.asarray(x, dtype=np.float32)
    W1S = np.asarray(W1S, dtype=np.float32)
    W2S = np.asarray(W2S, dtype=np.float32)
    W1NS = np.asarray(W1NS, dtype=np.float32)
    W2NS = np.asarray(W2NS, dtype=np.float32)

    # Shared weights: identical on every core.
    w1_pk = np.ascontiguousarray(
        W1S.reshape(KD, 128, W1G, W1GW).transpose(2, 1, 0, 3).astype(np.float16)
    )
    w2_pk = np.ascontiguousarray(
        W2S.reshape(MF, 128, NQ, 128).transpose(2, 1, 0, 3).astype(np.float16)
    )

    in_maps = []
    for c in range(NCORES):
        xt = np.ascontiguousarray(
            x[c, :S].T.reshape(KD, 128, S).transpose(1, 0, 2).astype(np.float16)
        )

        w1n = np.ascontiguousarray(
            W1NS[QPC * c : QPC * (c + 1)]
            .reshape(QPC, KD, 128, PPN1, PPW1)
            .transpose(0, 2, 3, 1, 4)
            .astype(np.float16)
        )
        w2n = np.ascontiguousarray(
            W2NS[QPC * c : QPC * (c + 1)]
            .reshape(QPC, MF, 128, PPN2, PPW2)
            .transpose(0, 2, 3, 1, 4)
            .astype(np.float16)
        )
        xpos = x[:, S + QPC * c : S + QPC * (c + 1), :]          # [B, QPC, D]
        xp = np.ascontiguousarray(
            xpos.transpose(2, 1, 0)
            .reshape(KD, 128, QPC * B)
            .transpose(1, 0, 2)
            .astype(np.float16)
        )
        in_maps.append(
            {"xt": xt, "w1": w1_pk, "w2": w2_pk, "w1n": w1n, "w2n": w2n, "xp": xp}
        )
    return in_maps


def unpack_outputs(results):
    """Assemble the full [B, T, D] output from the 8 per-core result maps."""
    out = np.empty((B, T, D), dtype=np.float32)
    for c in range(NCORES):
        yst = results[c]["yst"]          # [D, S] f16
        ytp = results[c]["ytp"]          # [B*QPC, D] f32
        out[c, :S, :] = yst.T.astype(np.float32)
        for q in range(QPC):
            out[:, S + QPC * c + q, :] = ytp[q * B : (q + 1) * B, :]
    return out


def kernel(x, W1S, W2S, W1NS, W2NS):
    from concourse.bass_utils import run_bass_kernel_spmd

    nc = _build_nc()
    in_maps = pack_inputs(x, W1S, W2S, W1NS, W2NS)
    res = run_bass_kernel_spmd(nc, in_maps, core_ids=list(range(NCORES)))
    return unpack_outputs(res.results)



# revision 10
# speedup vs baseline: 6.3211x; 6.3211x over previous
# Trainium2 Bass kernel for nn_MixedFFN (B=8, T=2048, D=1024, F=4096, LNS=32).
#
# Sharding across 8 NeuronCores (no collectives needed):
#   - Shared-FFN branch (tokens 0..2015): core c handles batch row c.
#   - Per-position branch (last 32 token positions, distinct weights per
#     position): core c handles positions 4c..4c+3 for all 8 batch rows.
#
# Per-core math:
#   shared: ysT = (gelu(x_c @ W1S) @ W2S).T computed as
#     L1: hT[f, t] = sum_k W1S[k, f] * xT[k, t]      (lhsT = W1S tile, rhs = xT)
#         a = gelu(h) stored transposed [F, tokens] in fp16
#     L2: ysT[d, t] = sum_f W2S[f, d] * aT[f, t]     (lhsT = W2S tile, rhs = aT)
#   per-position: the four positions' GEMMs run CONCURRENTLY on the PE via
#     column tiling (tile_position=(0, 32q)); each position's [8 x 512] psum
#     lives in its own bank at partition offset 32q.  gelu lands in app[128, F]
#     (rows 32q+b); one SBUF->SBUF DMA transpose produces aT for L2 (no PE
#     transposes).  pp weights (67MB/core) stream throughout the kernel,
#     interleaved with shared-branch compute via emission hooks.
#
# All matmuls in fp16 (PE streams 1 col/cycle), PSUM accumulates fp32.
# Shared tokens processed exactly (2016 = 2 blocks x 2 chunks x 504), no pad.
# yst output is fp16 (host upcasts); ytp stays fp32.

import numpy as np

B, T, D, F, LNS = 8, 2048, 1024, 4096, 32
S = T - LNS            # 2016 shared tokens per batch row
NCORES = 8
QPC = LNS // NCORES    # 4 positions per core
NBLK = 2               # token blocks for the shared branch
BLK = S // NBLK        # 1008 tokens per block
CH = BLK // 2          # 504-token chunks (PE moving dim <= 512)
KD = D // 128          # 8  k-tiles over D
MF = F // 128          # 32 m-tiles over F
W1G = 8                # W1S column groups (4 m-tiles = 512 cols each, 1MB)
W1GW = F // W1G        # 512
NQ = 8                 # D-column tiles for L2 output (dq)
PPN1 = 8               # per-position L1 chunks over F (512 wide)
PPW1 = F // PPN1       # 512
KDH = 4                # k-tiles per pp-L1 weight tile (16KB/partition tiles)
KG1 = KD // KDH        # 2 pp-L1 contraction groups
PPN2 = 2               # per-position L2 chunks over D (512 wide)
PPW2 = D // PPN2       # 512
KK2 = 4                # f-tiles per pp-L2 weight tile
KG2 = MF // KK2        # 8 pp-L2 contraction groups

_CACHE = {}


def _build_nc(loop_n=0, parts="all"):
    """Build + bacc-compile the single-core SPMD program. Cached per process.

    loop_n > 0 wraps the whole body in a hardware For_i loop that repeats the
    kernel loop_n times inside one NEFF execution — a timing instrument only.
    parts: "all" | "shared" | "pp" — emit only a subset (timing experiments).
    """
    key = ("nc", loop_n, parts)
    if key in _CACHE:
        return _CACHE[key]

    import concourse.mybir as mybir
    import concourse.tile as tile
    from concourse import bacc

    f32 = mybir.dt.float32
    f16 = mybir.dt.float16
    GELU = mybir.ActivationFunctionType.Gelu

    nc = bacc.Bacc(None, target_bir_lowering=False)

    # ---- kernel I/O (per-core shapes; host packs these layouts) ----
    xt_d = nc.dram_tensor("xt", [128, KD, S], f16, kind="ExternalInput")
    w1_d = nc.dram_tensor("w1", [W1G, 128, KD, W1GW], f16, kind="ExternalInput")
    w2_d = nc.dram_tensor("w2", [NQ, 128, MF, 128], f16, kind="ExternalInput")
    w1n_d = nc.dram_tensor(
        "w1n", [PPN1, KG1, 128, QPC, KDH, PPW1], f16, kind="ExternalInput"
    )
    w2n_d = nc.dram_tensor(
        "w2n", [PPN2, KG2, 128, QPC, KK2, PPW2], f16, kind="ExternalInput"
    )
    xp_d = nc.dram_tensor("xp", [128, KD, B * QPC], f16, kind="ExternalInput")
    yst_d = nc.dram_tensor("yst", [D, S], f16, kind="ExternalOutput")
    ytp_d = nc.dram_tensor("ytp", [128, PPN2, PPW2], f32, kind="ExternalOutput")

    with tile.TileContext(nc) as tc:
        with (
            tc.tile_pool(name="xqp", bufs=2) as xqp,        # [128, 8, 1008] f16, 2MB
            tc.tile_pool(name="w1p", bufs=2) as w1p,        # [128, 8, 512] f16, 1MB
            tc.tile_pool(name="atp", bufs=1) as atp,        # [128, 32, 1008] f16, 8.3MB
            tc.tile_pool(name="w2p", bufs=2) as w2p,        # [128, 32, 128] f16, 1MB
            tc.tile_pool(name="wnp", bufs=3) as wnp,        # [128, 4, 4, 512] f16, 1MB
            tc.tile_pool(name="xpp", bufs=1) as xpp,
            tc.tile_pool(name="appp", bufs=1) as appp,      # [128, 4096] f16, 8KB/part
            tc.tile_pool(name="at32p", bufs=1) as at32p,    # [128, 32, 128] f16
            tc.tile_pool(name="outp", bufs=3) as outp,      # [128, 504] f16
            tc.tile_pool(name="ypp", bufs=2) as ypp,        # [128, 512] f32
            tc.tile_pool(name="psum", bufs=2, space="PSUM") as psum,
        ):
            import contextlib

            loop_cm = (
                tc.For_i(0, loop_n, 1, hint_engines=(mybir.EngineType.PE,))
                if loop_n
                else contextlib.nullcontext()
            )
            with loop_cm:
              xp_holder = []
              app_holder = []
              at32_holder = []
              wn_tiles = {}
              pp_ps = {}

              def emit_xp():
                  xp_sb = xpp.tile([128, KD, B * QPC], f16, name="xp_sb")
                  xp_holder.append(xp_sb)
                  nc.sync.dma_start(out=xp_sb[:], in_=xp_d[:])
                  app = appp.tile([128, F], f16, name="app")
                  app_holder.append(app)
                  nc.vector.memset(app[:], 0.0)

              # ---------- per-position emission units ----------
              def ppl1_dma(nch, kg):
                  t = wnp.tile(
                      [128, QPC, KDH, PPW1], f16, name=f"w1n_{nch}_{kg}", tag="wn"
                  )
                  wn_tiles[("l1", nch, kg)] = t
                  nc.sync.dma_start(out=t[:], in_=w1n_d[nch, kg])

              def ppl1_mm(nch, kg):
                  xp_sb = xp_holder[0]
                  if kg == 0:
                      for q in range(QPC):
                          pp_ps[q] = psum.tile(
                              [128, PPW1], f32, tag=f"pp{q}", bufs=1,
                              name=f"psl1_{nch}_{q}",
                          )
                  t = wn_tiles.pop(("l1", nch, kg))
                  for kk in range(KDH):
                      k = kg * KDH + kk
                      for q in range(QPC):
                          nc.tensor.matmul(
                              pp_ps[q][32 * q : 32 * q + 8, :],
                              xp_sb[:, k, 8 * q : 8 * q + 8],
                              t[:, q, kk, :],
                              start=(k == 0),
                              stop=(k == KD - 1),
                              tile_position=(0, 32 * q),
                          )
                  if kg == KG1 - 1:
                      app = app_holder[0]
                      for q in range(QPC):
                          nc.scalar.activation(
                              app[32 * q : 32 * q + 8, nch * PPW1 : (nch + 1) * PPW1],
                              pp_ps[q][32 * q : 32 * q + 8, :],
                              GELU,
                          )

              def pp_transpose():
                  at32 = at32p.tile([128, MF, 128], f16, name="at32")
                  at32_holder.append(at32)
                  nc.sync.dma_start_transpose(out=at32[:], in_=app_holder[0][:])

              def ppl2_dma(nch2, kg):
                  t = wnp.tile(
                      [128, QPC, KK2, PPW2], f16, name=f"w2n_{nch2}_{kg}", tag="wn"
                  )
                  wn_tiles[("l2", nch2, kg)] = t
                  nc.sync.dma_start(out=t[:], in_=w2n_d[nch2, kg])

              def ppl2_mm(nch2, kg):
                  at32 = at32_holder[0]
                  if kg == 0:
                      for q in range(QPC):
                          pp_ps[q] = psum.tile(
                              [128, PPW2], f32, tag=f"pp{q}", bufs=1,
                              name=f"psl2_{nch2}_{q}",
                          )
                  t = wn_tiles.pop(("l2", nch2, kg))
                  for kk in range(KK2):
                      k = kg * KK2 + kk
                      for q in range(QPC):
                          nc.tensor.matmul(
                              pp_ps[q][32 * q : 32 * q + 8, :],
                              at32[:, k, 32 * q : 32 * q + 8],
                              t[:, q, kk, :],
                              start=(k == 0),
                              stop=(k == MF - 1),
                              tile_position=(0, 32 * q),
                          )

              def ppl2_out(nch2):
                  yt = ypp.tile([128, PPW2], f32, name=f"yt_{nch2}", tag="yt")
                  for q in range(QPC):
                      nc.vector.tensor_copy(
                          yt[32 * q : 32 * q + 8, :],
                          pp_ps[q][32 * q : 32 * q + 8, :],
                      )
                      nc.sync.dma_start(
                          out=ytp_d[32 * q : 32 * q + 8, nch2, :],
                          in_=yt[32 * q : 32 * q + 8, :],
                      )

              # side unit lists: software-pipelined (DMA 2 tiles ahead of MMs)
              aseq = [(nch, kg) for nch in range(PPN1) for kg in range(KG1)]
              sideA = [emit_xp,
                       lambda: ppl1_dma(*aseq[0]), lambda: ppl1_dma(*aseq[1])]
              for i, (nch, kg) in enumerate(aseq):
                  if i + 2 < len(aseq):
                      sideA.append(lambda a=aseq[i + 2]: ppl1_dma(*a))
                  sideA.append(lambda n=nch, kg=kg: ppl1_mm(n, kg))

              dseq = [(n2, kg) for n2 in range(PPN2) for kg in range(KG2)]
              sideB = [pp_transpose,
                       lambda: ppl2_dma(*dseq[0]), lambda: ppl2_dma(*dseq[1])]
              for i, (n2, kg) in enumerate(dseq):
                  if i + 2 < len(dseq):
                      sideB.append(lambda a=dseq[i + 2]: ppl2_dma(*a))
                  sideB.append(lambda n2=n2, kg=kg: ppl2_mm(n2, kg))
                  if kg == KG2 - 1:
                      sideB.append(lambda n2=n2: ppl2_out(n2))

              # ---------- shared-branch emission units ----------
              def shared_l1_block(blk):
                  at_b = atp.tile([128, MF, BLK], f16, name=f"at_{blk}", tag="at")
                  w1g0 = w1p.tile([128, KD, W1GW], f16, name=f"w1g_{blk}_0", tag="w1g")
                  nc.sync.dma_start(out=w1g0[:], in_=w1_d[0])
                  xq = xqp.tile([128, KD, BLK], f16, name=f"xq_{blk}", tag="xq")
                  for tch in range(2):
                      nc.sync.dma_start(
                          out=xq[:, :, tch * CH : (tch + 1) * CH],
                          in_=xt_d[:, :, blk * BLK + tch * CH : blk * BLK + (tch + 1) * CH],
                      )

                  def unit(g, hook):
                      if g == 0:
                          w1g = w1g0
                      else:
                          w1g = w1p.tile([128, KD, W1GW], f16, name=f"w1g_{blk}_{g}", tag="w1g")
                          nc.sync.dma_start(out=w1g[:], in_=w1_d[g])
                      for ml in range(4):
                          m = 4 * g + ml
                          for tch in range(2):
                              ps = psum.tile([128, CH], f32, tag="l1", bufs=2, name=f"psl1s_{blk}_{m}_{tch}")
                              for k in range(KD):
                                  nc.tensor.matmul(
                                      ps[:],
                                      w1g[:, k, ml * 128 : (ml + 1) * 128],
                                      xq[:, k, tch * CH : (tch + 1) * CH],
                                      start=(k == 0),
                                      stop=(k == KD - 1),
                                  )
                              nc.scalar.activation(
                                  at_b[:, m, tch * CH : (tch + 1) * CH], ps[:], GELU
                              )
                              hook(1)

                  return at_b, [lambda hook, g=g: unit(g, hook) for g in range(W1G)]

              def shared_l2_unit(blk, at_b, dq, hook):
                  w2g = w2p.tile([128, MF, 128], f16, name=f"w2g_{blk}_{dq}", tag="w2g")
                  nc.sync.dma_start(out=w2g[:], in_=w2_d[dq])
                  for tch in range(2):
                      ps = psum.tile([128, CH], f32, tag="l2", bufs=2, name=f"psl2s_{blk}_{dq}_{tch}")
                      for k in range(MF):
                          nc.tensor.matmul(
                              ps[:],
                              w2g[:, k, :],
                              at_b[:, k, tch * CH : (tch + 1) * CH],
                              start=(k == 0),
                              stop=(k == MF - 1),
                          )
                      ot = outp.tile([128, CH], f16, name=f"ot_{blk}_{dq}_{tch}", tag="ot")
                      nc.vector.tensor_copy(ot[:], ps[:])
                      nc.sync.dma_start(
                          out=yst_d[
                              dq * 128 : (dq + 1) * 128,
                              blk * BLK + tch * CH : blk * BLK + (tch + 1) * CH,
                          ],
                          in_=ot[:],
                      )
                      hook(4)

              # ---------- emission schedule ----------
              # pp units pace DMA-bandwidth-uniformly across each shared block:
              # L1 psum groups weight 1 (~1.7us), L2 groups weight 4 (~6.7us).
              if parts in ("all", "shared"):
                  side = None
                  budget = [0.0]
                  side_i = [0]

                  def emit_side(n):
                      while side_i[0] < len(side) and n > 0:
                          side[side_i[0]]()
                          side_i[0] += 1
                          n -= 1

                  def hook(w):
                      if side is None or parts == "shared":
                          return
                      budget[0] += w * len(side) / 128.0
                      while budget[0] >= 1.0 and side_i[0] < len(side):
                          side[side_i[0]]()
                          side_i[0] += 1
                          budget[0] -= 1.0

                  for blk in range(NBLK):
                      if parts == "all":
                          side = sideA if blk == 0 else sideB
                          side_i = [0]
                          budget = [2.0 if blk == 0 else 2.0]  # front-load prologue
                      at_b, l1_units = shared_l1_block(blk)
                      for u in l1_units:
                          u(hook)
                      for dq in range(NQ):
                          shared_l2_unit(blk, at_b, dq, hook)
                      if parts == "all":
                          emit_side(len(side))
              else:
                  for u in sideA:
                      u()
                  for u in sideB:
                      u()

    nc.compile()
    _CACHE[key] = nc
    return nc


def pack_inputs(x, W1S, W2S, W1NS, W2NS):
    """Build the 8 per-core input maps (numpy, host-side layout packing)."""
    x = np.asarray(x, dtype=np.float32)
    W1S = np.asarray(W1S, dtype=np.float32)
    W2S = np.asarray(W2S, dtype=np.float32)
    W1NS = np.asarray(W1NS, dtype=np.float32)
    W2NS = np.asarray(W2NS, dtype=np.float32)

    # Shared weights: identical on every core.
    w1_pk = np.ascontiguousarray(
        W1S.reshape(KD, 128, W1G, W1GW).transpose(2, 1, 0, 3).astype(np.float16)
    )
    w2_pk = np.ascontiguousarray(
        W2S.reshape(MF, 128, NQ, 128).transpose(2, 1, 0, 3).astype(np.float16)
    )

    in_maps = []
    for c in range(NCORES):
        xt = np.ascontiguousarray(
            x[c, :S].T.reshape(KD, 128, S).transpose(1, 0, 2).astype(np.float16)
        )

        # w1n[nch, kg, p, q, kk, j] = W1NS[4c+q, (kg*4+kk)*128+p, nch*512+j]
        w1n = np.ascontiguousarray(
            W1NS[QPC * c : QPC * (c + 1)]
            .reshape(QPC, KG1, KDH, 128, PPN1, PPW1)
            .transpose(4, 1, 3, 0, 2, 5)
            .astype(np.float16)
        )
        # w2n[nch2, kg, p, q, kk, j] = W2NS[4c+q, (kg*4+kk)*128+p, nch2*512+j]
        w2n = np.ascontiguousarray(
            W2NS[QPC * c : QPC * (c + 1)]
            .reshape(QPC, KG2, KK2, 128, PPN2, PPW2)
            .transpose(4, 1, 3, 0, 2, 5)
            .astype(np.float16)
        )
        xpos = x[:, S + QPC * c : S + QPC * (c + 1), :]          # [B, QPC, D]
        xp = np.ascontiguousarray(
            xpos.transpose(2, 1, 0)
            .reshape(KD, 128, QPC * B)
            .transpose(1, 0, 2)
            .astype(np.float16)
        )
        in_maps.append(
            {"xt": xt, "w1": w1_pk, "w2": w2_pk, "w1n": w1n, "w2n": w2n, "xp": xp}
        )
    return in_maps


def unpack_outputs(results):
    """Assemble the full [B, T, D] output from the 8 per-core result maps."""
    out = np.empty((B, T, D), dtype=np.float32)
    for c in range(NCORES):
        yst = results[c]["yst"]          # [D, S] f16
        ytp = results[c]["ytp"]          # [128, PPN2, 512] f32 (rows 32q+b)
        out[c, :S, :] = yst.T.astype(np.float32)
        for q in range(QPC):
            for n2 in range(PPN2):
                out[:, S + QPC * c + q, n2 * PPW2 : (n2 + 1) * PPW2] = ytp[
                    32 * q : 32 * q + 8, n2, :
                ]
    return out


def kernel(x, W1S, W2S, W1NS, W2NS):
    from concourse.bass_utils import run_bass_kernel_spmd

    nc = _build_nc()
    in_maps = pack_inputs(x, W1S, W2S, W1NS, W2NS)
    res = run_bass_kernel_spmd(nc, in_maps, core_ids=list(range(NCORES)))
    return unpack_outputs(res.results)
